# revision 24
# baseline (speedup 1.0000x reference)
"""Masked weighted-NLL loss kernel for TRN2 — v13.

Dataflow: gather only the valid (mask=1) scores elements (offsets computed
host-side from the small gt/lengths tensors; all score values still move
through the device gather), balanced across 8 cores, then
Ln -> *mw -> ones-matmul -> reduce -> store; partials summed on host.
The compute chain for all but the last gather column is pipelined under
the last gather's drain (shared g_sem vs dedicated glast_sem), and the
last column's pad slots carry sentinel offsets above a bounds_check
register so they emit no descriptors (g is pre-memset to 1.0: ln(1)=0).

Schedule notes (from v6/v7/v8 traces):
  * vector-indirect DMA processes exactly 128 scattered addresses per
    instruction (~1.1us Q7 descriptor-gen each, strictly serialized), so
    instruction count = ceil(max_valid_per_core/128) — compaction +
    balancing takes 12 -> 7 for typical lengths;
  * the offset-table load goes on Sync's HWDGE (SWDGE completion receipt
    is ~1.7us vs ~0.85us), and offsets MUST be in SBUF (walrus:
    "Vector-dynamic-offsets location must be SB");
  * gathers alternate the two SWDGE queues, each with a completion sem
    (walrus requires a sync update on every DMA);
  * a warmup gather runs while the pk load is in flight (finishes before
    the pk sem fires, so it is free): it absorbs the cold-SWDGE first-op
    cost, and skipping it showed a rare nondeterministic NaN;
  * default Block drain kept: no_gpsimd_drain=True also showed a flaky run.
"""

import numpy as np

B, T, V = 64, 188, 32000
N_CORES = 8
BETA = 2.0
P = 128

_NC_CACHE = {}


def _build_nc(ncol, nrows, detect_races=True, bounds_skip=True):
    import concourse.bacc as bacc
    import concourse.bass as bass
    import concourse.mybir as mybir

    nc = bacc.Bacc(
        "TRN2",
        target_bir_lowering=False,
        debug=False,
        num_swdge_queues=2,
        detect_race_conditions=detect_races,
    )

    def indirect_gather(out, in_, off_ap, queue, bounds_reg=None):
        """HW semantics: out[p, 0] = in_[off_ap[p, 0]] (one scattered
        address per partition). With bounds set, offsets > bounds are
        silently skipped (no descriptor, no write) — pad slots use a
        huge sentinel offset and keep their pre-memset value."""
        gp = nc.gpsimd
        out_l = gp.lower_ap_dma(out, for_indirect_dma=True)
        in_l = gp.lower_ap_dma(in_, for_indirect_dma=True)
        assert len(in_l) == 1 and len(out_l) == 1
        off_l = gp.lower_ap_dma(off_ap)
        assert len(off_l) == 1
        in_l.append(off_l[0])
        in_l[0].dynamic_ap_info = mybir.DynamicAccessPatternInfo(
            c=0,
            actual_ap=out.ap,
            indirect_dim_max_index=in_.shape[0],
            offset_expr=[
                mybir.DynamicAccessPatternOffsetExpr(
                    coef=1,
                    aff_expr=mybir.DynamicAccessPatternOffsetExprAffExpr(
                        kind="IndirectArgId", arg_id=1
                    ),
                )
            ],
        )
        bc = [] if bounds_reg is None else [gp.lower_val_access(bounds_reg)]
        return gp.add_instruction(
            mybir.InstDMACopy(
                name=nc.get_next_instruction_name(),
                queue=queue,
                mode="Copy",
                ins=in_l + bc,
                outs=out_l,
                oob_is_err=bounds_reg is None,
                cce_op=mybir.AluOpType.bypass,
            )
        )

    scores = nc.dram_tensor(
        "scores", [nrows * V, 1], mybir.dt.float32, kind="ExternalInput"
    )
    pk_d = nc.dram_tensor("pk", [P, 2 * ncol], mybir.dt.int32, kind="ExternalInput")
    out = nc.dram_tensor("out", [1, 1], mybir.dt.float32, kind="ExternalOutput")

    f32 = mybir.dt.float32
    i32 = mybir.dt.int32
    Alu = mybir.AluOpType
    Ln = mybir.ActivationFunctionType.Ln

    def full(t):
        sh = t.shape
        return bass.AP(t, 0, [[sh[1], sh[0]], [1, sh[1]]])

    def col(t, j0, n):
        sh = t.shape
        return bass.AP(t, j0, [[sh[1], sh[0]], [1, n]])

    with (
        nc.semaphore("ones_sem") as ones_sem,
        nc.semaphore("warm_sem") as warm_sem,
        nc.semaphore("wg_sem") as wg_sem,
        nc.semaphore("load_sem") as load_sem,
        nc.semaphore("g_sem") as g_sem,
        nc.semaphore("go_sem") as go_sem,
        nc.semaphore("gset_sem") as gset_sem,
        nc.semaphore("act_sem") as act_sem,
        nc.semaphore("tt_sem") as tt_sem,
        nc.semaphore("mm_sem") as mm_sem,
        nc.semaphore("red_sem") as red_sem,
        nc.semaphore("out_sem") as out_sem,
        nc.sbuf_tensor([P, 1], f32) as ones,
        nc.sbuf_tensor([P, 1], f32) as scratch,
        nc.sbuf_tensor([P, 1], i32) as woffs,
        nc.sbuf_tensor([P, 1], f32) as wg,
        nc.sbuf_tensor([P, 2 * ncol], i32) as pk,
        nc.sbuf_tensor([P, ncol], f32) as g,
        nc.sbuf_tensor([P, ncol], f32) as logg,
        nc.sbuf_tensor([P, ncol], f32) as prod,
        nc.psum_tensor([1, ncol], f32) as colsum,
        nc.sbuf_tensor([1, 1], f32) as res,
    ):
        with nc.Block() as block:

            @block.sync
            def _(sync):
                sync.dma_start(full(pk), full(pk_d)).then_inc(load_sem, 16)

            @block.vector
            def _(vector):
                vector.memset(full(ones), 1.0).then_inc(ones_sem, 1)
                vector.memset(full(g), 1.0).then_inc(gset_sem, 1)

            @block.scalar
            def _(scalar):
                # dummy Ln: hoists the compiler's ACT_TABLE_LOAD off the
                # critical path (runs during the pk load / gather train)
                scalar.wait_ge(ones_sem, 1)
                scalar.activation(full(scratch), full(ones), Ln)

            @block.gpsimd
            def _(gpsimd):
                # warmup: dummy gather from scores[0]*128 while the pk load
                # is in flight — absorbs the cold-SWDGE first-op cost
                gpsimd.memset(full(woffs), 0).then_inc(warm_sem, 1)
                gpsimd.wait_ge(warm_sem, 1)
                indirect_gather(
                    full(wg), full(scores), full(woffs), "qPoolDynamic"
                ).then_inc(wg_sem, 16)
                breg = (
                    gpsimd.to_reg(nrows * V - 1) if bounds_skip else None
                )
                gpsimd.wait_ge(load_sem, 16)
                gpsimd.wait_ge(gset_sem, 1)
                for j in range(ncol):
                    indirect_gather(
                        col(g, j, 1),
                        full(scores),
                        col(pk, j, 1),
                        "qPoolDynamic" if j % 2 == 0 else "qPoolDynamic1",
                        bounds_reg=breg if j == ncol - 1 else None,
                    ).then_inc(g_sem if j < ncol - 1 else glast_sem, 16)

            # split the chain: cols [0, sp) carry g_sem (16 each, any
            # order) and process under the last gather's drain window;
            # col sp carries glast_sem and gates only the short final chain
            sp = ncol - 1

            @block.scalar
            def _(scalar):
                if sp:
                    scalar.wait_ge(g_sem, 16 * sp)
                    scalar.activation(
                        col(logg, 0, sp), col(g, 0, sp), Ln
                    ).then_inc(act_sem, 1)
                scalar.wait_ge(glast_sem, 16)
                scalar.activation(
                    col(logg, sp, 1), col(g, sp, 1), Ln
                ).then_inc(act_sem, 1)

            @block.vector
            def _(vector):
                if sp:
                    vector.wait_ge(act_sem, 1)
                    vector.tensor_tensor(
                        out=col(prod, 0, sp),
                        in0=col(logg, 0, sp),
                        in1=col(pk, ncol, sp).bitcast(f32),
                        op=Alu.mult,
                    ).then_inc(tt_sem, 1)
                vector.wait_ge(act_sem, 2 if sp else 1)
                vector.tensor_tensor(
                    out=col(prod, sp, 1),
                    in0=col(logg, sp, 1),
                    in1=col(pk, ncol + sp, 1).bitcast(f32),
                    op=Alu.mult,
                ).then_inc(tt_sem, 1)

            @block.tensor
            def _(tensor):
                if sp:
                    tensor.wait_ge(tt_sem, 1)
                    tensor.matmul(
                        col(colsum, 0, sp), full(ones), col(prod, 0, sp),
                        start=True, stop=True,
                    ).then_inc(mm_sem, 1)
                tensor.wait_ge(tt_sem, 2 if sp else 1)
                tensor.matmul(
                    col(colsum, sp, 1), full(ones), col(prod, sp, 1),
                    start=True, stop=True,
                ).then_inc(mm_sem, 1)

            @block.vector
            def _(vector):
                vector.wait_ge(mm_sem, 2 if sp else 1)
                vector.reduce_sum(
                    out=full(res), in_=full(colsum), axis=mybir.AxisListType.X
                ).then_inc(red_sem, 1)

            @block.sync
            def _(sync):
                # no completion wait on the store: the framework epilogue's
                # drain guarantees the write lands before the NEFF completes
                sync.wait_ge(red_sem, 1)
                sync.dma_start(full(out), full(res)).then_inc(out_sem, 16)

    nc.compile()
    return nc


def _plan(targets_ground_truth, lengths):
    """Balanced compact shard plan from the small inputs (index math only).

    Returns (ncol, nrows, chunks) where chunks[c] = (k0, rows, offs, mw):
    k0/rows give the contiguous row-range of flattened scores core c needs;
    offs/mw are the [P, ncol] slot tables (offsets relative to k0*V).
    """
    gt = np.ascontiguousarray(targets_ground_truth).astype(np.int64).reshape(B * T)
    ln = np.ascontiguousarray(lengths).astype(np.int64)

    tt = np.arange(B * T) % T
    bb = np.arange(B * T) // T
    valid = tt < ln[bb]
    vk = np.nonzero(valid)[0]            # sorted flattened (b,t) indices
    n = len(vk)
    m = max(1, (n + N_CORES - 1) // N_CORES)
    ncol = max(1, (m + P - 1) // P)

    chunks = []
    nrows = 1
    for c in range(N_CORES):
        ks = vk[c * m : (c + 1) * m]
        offs = np.zeros((P, ncol), dtype=np.int32)
        offs[:, ncol - 1] = 0x7FFFFFF0  # skip sentinel, last col only
        mw = np.zeros((P, ncol), dtype=np.float32)
        if len(ks):
            k0 = int(ks[0])
            rows = int(ks[-1]) - k0 + 1
            nrows = max(nrows, rows)
            i = np.arange(len(ks))
            p, j = i % P, i // P
            w = np.where(gt[ks] == 0, 1.0, BETA)
            offs[p, j] = ((ks - k0) * V + gt[ks]).astype(np.int32)
            mw[p, j] = (-w / B).astype(np.float32)
        else:
            k0, rows = 0, 1
        chunks.append((k0, rows, offs, mw))
    return ncol, nrows, chunks


def _shard_inputs(targets_scores, targets_ground_truth, lengths):
    ncol, nrows, chunks = _plan(targets_ground_truth, lengths)
    flat = np.ascontiguousarray(targets_scores, dtype=np.float32).reshape(B * T, V)
    in_maps = []
    for k0, rows, offs, mw in chunks:
        s = np.zeros((nrows * V, 1), dtype=np.float32)
        s[: rows * V, 0] = flat[k0 : k0 + rows].reshape(-1)
        if s[0, 0] == 0.0:
            # pad slots gather offset 0 with weight 0; keep ln() finite
            s[0, 0] = 1.0
        pk = np.concatenate([offs, mw.view(np.int32)], axis=1)  # [P, 2*ncol]
        in_maps.append({"scores": s, "pk": np.ascontiguousarray(pk)})
    return (ncol, nrows), in_maps


def _partial_f64(in_map):
    """Host reference for one core's partial sum (used by sim checks)."""
    s = in_map["scores"].reshape(-1).astype(np.float64)
    ncol = in_map["pk"].shape[1] // 2
    offs = in_map["pk"][:, :ncol].astype(np.int64)
    mw = in_map["pk"][:, ncol:].view(np.float32).astype(np.float64)
    offs = np.minimum(offs, len(s) - 1)
    return np.sum(mw * np.log(np.where(mw != 0, s[offs], 1.0)))


def _run(targets_scores, targets_ground_truth, lengths, trace=False, **spmd_kwargs):
    from concourse.bass_utils import run_bass_kernel_spmd

    key, in_maps = _shard_inputs(targets_scores, targets_ground_truth, lengths)
    if key not in _NC_CACHE:
        _NC_CACHE[key] = _build_nc(*key)
    return run_bass_kernel_spmd(
        _NC_CACHE[key],
        in_maps,
        core_ids=list(range(N_CORES)),
        trace=trace,
        **spmd_kwargs,
    )


def kernel(targets_scores, targets_ground_truth, lengths):
    r = _run(targets_scores, targets_ground_truth, lengths)
    total = np.sum(
        [np.sum(res["out"], dtype=np.float64) for res in r.results], dtype=np.float64
    )
    return np.array([total], dtype=np.float32)
